# revision 1
# baseline (speedup 1.0000x reference)
"""Trainium2 Bass kernel for nn_CritiGraph.

Math (bitwise-exact vs the fp32 reference):
  dist(c1,c2,n) = sg * (1 - e/16) * n,  sg = sign(c1)*sign(c2),
  e = frexp_exp(|c1|^|c2| + 1) = bexp(float(|c1|^|c2|) + 1.5) - 126.
  ct[t,s,c,tp] = Q[t,s,tp] + M1[t,s,tp] * R[t,s,c,tp]
  where R = sgc * (e-16) (cnc sign applied via bf16 sign-bit xor),
        M1 = -sgp * norm / 128  (pos sign + /TP/16 folded),
        Q  = (sum_tp g - g)/8,  g = cos_sta_pos.

Sharding: T=128 rows split across 8 cores (16 rows each). Inputs are full
tensors; each core receives its T-slice. Output gathered on host.
"""
import dataclasses
import numpy as np

import concourse.bass as bass
import concourse.mybir as mybir
from concourse import tile, bacc
from concourse.bass_utils import run_bass_kernel_spmd

dt = mybir.dt
Alu = mybir.AluOpType
Act = mybir.ActivationFunctionType

T, S, TP, C = 128, 128, 8, 257
NCORES = 8
TL = T // NCORES          # 16 t-rows per core
CP = C + 1                # padded c (even FD for DVE perf modes)
FDP = TP * CP             # 2064, tp-major free width
FDO = C * TP              # 2056, output free width

# engine split for P5: which tp slices go to ACT (rest on DVE)
P5_ACT_TPS = (4, 5, 6, 7)
# engine for P3b convert: 'gpsimd' | 'vector' | 'scalar'
P3B_ENGINE = "gpsimd"


def _rep128(ap_row):
    """DRAM row AP -> same row broadcast to 128 partitions (stride-0)."""
    return dataclasses.replace(ap_row, ap=[[0, 128]] + list(ap_row.ap)[1:])


def build_nc(repeat=1, repeat_loop=1, *, skip_repl=False, p2_engine="scalar",
             p3b_engine=None, p5_act_tps=None, skip_stages=()):
    p3b_engine = p3b_engine or P3B_ENGINE
    p5_act_tps = P5_ACT_TPS if p5_act_tps is None else p5_act_tps
    nc = bacc.Bacc("TRN2", target_bir_lowering=False, debug=False)

    sta_d = nc.dram_tensor("sta_loc", [TL, TP], dt.int32, kind="ExternalInput")
    pos_d = nc.dram_tensor("pos_loc", [TL, S, TP], dt.int32, kind="ExternalInput")
    cnc_d = nc.dram_tensor("cnc_loc", [TL, C, TP], dt.int32, kind="ExternalInput")
    norm_d = nc.dram_tensor("eu_norm", [TL, S], dt.float32, kind="ExternalInput")
    ct_d = nc.dram_tensor("ct", [TL, S, C, TP], dt.float32, kind="ExternalOutput")

    with tile.TileContext(nc) as tc:
        with (
            tc.tile_pool(name="const", bufs=1) as cpool,
            tc.tile_pool(name="work", bufs=3) as wpool,
            tc.tile_pool(name="dram", bufs=1, space="DRAM") as dpool,
        ):
            # ---------------- preprocessing (small) ----------------
            cncraw = cpool.tile([TL, C * TP], dt.int32)
            nc.sync.dma_start(cncraw[:], cnc_d[:].rearrange("t c p -> t (c p)"))

            # tp-major magnitude / sign planes, padded to CP per tp block
            c_mag16 = cpool.tile([TL, FDP], dt.uint16)
            c_sgn16 = cpool.tile([TL, FDP], dt.uint16)
            nc.vector.memset(c_mag16[:], 0)
            nc.vector.memset(c_sgn16[:], 0)
            # read (c,tp) natural; write offset tp*CP + c
            cnc_r = cncraw[:].rearrange("t (c p) -> t c p", p=TP)
            cmag_w = dataclasses.replace(
                c_mag16[:], ap=[list(c_mag16[:].ap)[0], [1, C], [CP, TP]])
            csgn_w = dataclasses.replace(
                c_sgn16[:], ap=[list(c_sgn16[:].ap)[0], [1, C], [CP, TP]])
            nc.scalar.activation(cmag_w, cnc_r, Act.Abs)
            nc.vector.tensor_scalar(csgn_w, cnc_r, 0.0, 32768.0, Alu.is_lt, Alu.mult)

            # stage to DRAM for per-t replicated reads
            d_cmag = dpool.tile([TL, FDP], dt.uint16)
            d_csgn = dpool.tile([TL, FDP], dt.uint16)
            nc.sync.dma_start(d_cmag[:], c_mag16[:])
            nc.sync.dma_start(d_csgn[:], c_sgn16[:])

            # pos: [s, (t,tp)]
            posraw = cpool.tile([S, TL * TP], dt.int32)
            for t in range(TL):
                nc.sync.dma_start(posraw[:, t * TP:(t + 1) * TP], pos_d[t])
            p_mag16 = cpool.tile([S, TL * TP], dt.uint16)
            nc.scalar.activation(p_mag16[:], posraw[:], Act.Abs)

            # norm[s, t] via strided DMA (4B gather, 8KB once)
            norm_sb = cpool.tile([S, TL], dt.float32)
            norm_src = dataclasses.replace(
                norm_d[:].flatten(), ap=[[1, S], [S, TL]])
            nc.sync.dma_start(norm_sb[:], norm_src)
            normB = dataclasses.replace(
                norm_sb[:], ap=[list(norm_sb[:].ap)[0], [1, TL], [0, TP]])

            # M1[s,(t,tp)] = -sgp*norm/128 (exact: +-1/128 * norm)
            sgp2 = cpool.tile([S, TL * TP], dt.float32)
            nc.vector.tensor_scalar(sgp2[:], posraw[:], 0.0, 2.0, Alu.is_lt, Alu.mult)
            sgpm = cpool.tile([S, TL * TP], dt.float32)
            nc.vector.tensor_scalar(sgpm[:], sgp2[:], 1.0, 1.0 / 128, Alu.subtract, Alu.mult)
            M1 = cpool.tile([S, TL * TP], dt.float32)
            nc.vector.tensor_tensor(
                M1[:].rearrange("s (t p) -> s t p", p=TP), sgpm[:].rearrange("s (t p) -> s t p", p=TP),
                normB, Alu.mult)

            # sta replicated to all partitions: [s, (t,tp)]
            starep = cpool.tile([S, TL * TP], dt.int32)
            sta_src = dataclasses.replace(
                sta_d[:].flatten(), ap=[[0, S], [1, TL * TP]])
            nc.sync.dma_start(starep[:], sta_src)

            # g path -> Q
            stamag = cpool.tile([S, TL * TP], dt.uint16)
            nc.scalar.activation(stamag[:], starep[:], Act.Abs)
            zg16 = cpool.tile([S, TL * TP], dt.uint16)
            nc.vector.tensor_tensor(zg16[:], stamag[:], p_mag16[:], Alu.bitwise_xor)
            wg = cpool.tile([S, TL * TP], dt.float32)
            nc.scalar.activation(wg[:], zg16[:], Act.Copy, bias=1.5, scale=1.0)
            eg32 = cpool.tile([S, TL * TP], dt.int32)
            nc.vector.tensor_scalar(eg32[:], wg[:].bitcast(dt.int32), 23, None,
                                    Alu.logical_shift_right)
            sag = cpool.tile([S, TL * TP], dt.bfloat16)
            nc.vector.tensor_scalar(sag[:], eg32[:], 142.0, None, Alu.subtract)
            sgxp = cpool.tile([S, TL * TP], dt.int32)
            nc.vector.tensor_tensor(sgxp[:], starep[:], posraw[:], Alu.bitwise_xor)
            sgx32 = cpool.tile([S, TL * TP], dt.int32)
            nc.vector.tensor_scalar(sgx32[:], sgxp[:], 16, 0x8000,
                                    Alu.logical_shift_right, Alu.bitwise_and)
            sgx16 = cpool.tile([S, TL * TP], dt.uint16)
            nc.vector.tensor_scalar(sgx16[:], sgx32[:], 1.0, None, Alu.mult)
            rg16 = cpool.tile([S, TL * TP], dt.uint16)
            nc.vector.tensor_tensor(rg16[:], sag[:].bitcast(dt.uint16), sgx16[:],
                                    Alu.bitwise_xor)
            t1 = cpool.tile([S, TL * TP], dt.float32)
            nc.vector.tensor_tensor(
                t1[:].rearrange("s (t p) -> s t p", p=TP),
                rg16[:].bitcast(dt.bfloat16).rearrange("s (t p) -> s t p", p=TP),
                normB, Alu.mult)
            t2 = cpool.tile([S, TL], dt.float32)
            nc.vector.tensor_reduce(t2[:].unsqueeze(2),
                                    t1[:].rearrange("s (t p) -> s t p", p=TP),
                                    axis=mybir.AxisListType.X, op=Alu.add)
            t2s = cpool.tile([S, TL], dt.float32)
            nc.vector.tensor_scalar(t2s[:], t2[:], 1.0 / 128, None, Alu.mult)
            Q = cpool.tile([S, TL * TP], dt.float32)
            nc.vector.scalar_tensor_tensor(
                Q[:].rearrange("s (t p) -> s t p", p=TP),
                t1[:].rearrange("s (t p) -> s t p", p=TP),
                1.0 / 128,
                t2s[:].unsqueeze(2).to_broadcast([S, TL, TP]),
                Alu.mult, Alu.subtract)

            # ---------------- main loop over t ----------------
            def _main_body():
              for t in [tt for _ in range(repeat) for tt in range(TL)]:
                cncR = wpool.tile([S, FDP], dt.uint16, tag="cncR")
                scR = wpool.tile([S, FDP], dt.uint16, tag="scR")
                if not skip_repl:
                    nc.sync.dma_start(cncR[:], _rep128(d_cmag[t:t + 1, :]))
                    nc.sync.dma_start(scR[:], _rep128(d_csgn[t:t + 1, :]))

                z16 = wpool.tile([S, FDP], dt.uint16, tag="z16")
                if "p1" not in skip_stages:
                    for tp in range(TP):
                        nc.vector.tensor_scalar(
                            z16[:, tp * CP:(tp + 1) * CP],
                            cncR[:, tp * CP:(tp + 1) * CP],
                            p_mag16[:, t * TP + tp: t * TP + tp + 1],
                            None, Alu.bitwise_xor)

                w1 = wpool.tile([S, FDP], dt.float32, tag="w1")
                if "p2" not in skip_stages:
                    if p2_engine == "scalar":
                        nc.scalar.activation(w1[:], z16[:], Act.Copy, bias=1.5, scale=1.0)
                    else:
                        nc.vector.tensor_scalar(w1[:], z16[:], 1.5, None, Alu.add)

                e32 = wpool.tile([S, FDP], dt.int32, tag="e32")
                if "p3a" not in skip_stages:
                    nc.vector.tensor_scalar(e32[:], w1[:].bitcast(dt.int32), 23, None,
                                            Alu.logical_shift_right)
                sa = wpool.tile([S, FDP], dt.bfloat16, tag="sa")
                if "p3b" not in skip_stages:
                    eng = {"gpsimd": nc.gpsimd, "vector": nc.vector, "scalar": None}[p3b_engine]
                    if eng is None:
                        nc.scalar.activation(sa[:], e32[:], Act.Identity, bias=-142.0, scale=1.0)
                    else:
                        eng.tensor_scalar(sa[:], e32[:], 142.0, None, Alu.subtract)

                r16 = wpool.tile([S, FDP], dt.uint16, tag="r16")
                if "p4" not in skip_stages:
                    nc.vector.tensor_tensor(r16[:], sa[:].bitcast(dt.uint16), scR[:],
                                            Alu.bitwise_xor)

                out_sb = wpool.tile([S, C, TP], dt.float32, tag="out")
                for tp in (() if "p5" in skip_stages else range(TP)):
                    rsl = r16[:, tp * CP: tp * CP + C].bitcast(dt.bfloat16)
                    m1c = M1[:, t * TP + tp: t * TP + tp + 1]
                    qc = Q[:, t * TP + tp: t * TP + tp + 1]
                    if tp in p5_act_tps:
                        nc.scalar.activation(out_sb[:, :, tp], rsl, Act.Identity,
                                             bias=qc, scale=m1c)
                    else:
                        nc.vector.tensor_scalar(out_sb[:, :, tp], rsl, m1c, qc,
                                                Alu.mult, Alu.add)

                nc.sync.dma_start(ct_d[t], out_sb[:])

            if repeat_loop > 1:
                with tc.For_i(0, repeat_loop, 1):
                    _main_body()
            else:
                _main_body()

    nc.compile()
    return nc


_NC_CACHE = None


def kernel(sta_loc, pos_loc, cnc_loc, eu_norm):
    global _NC_CACHE
    if _NC_CACHE is None:
        _NC_CACHE = build_nc()
    nc = _NC_CACHE

    sta_loc = np.ascontiguousarray(np.asarray(sta_loc, dtype=np.int32))
    pos_loc = np.ascontiguousarray(np.asarray(pos_loc, dtype=np.int32))
    cnc_loc = np.ascontiguousarray(np.asarray(cnc_loc, dtype=np.int32))
    eu_norm = np.ascontiguousarray(np.asarray(eu_norm, dtype=np.float32))

    in_maps = []
    for c in range(NCORES):
        lo, hi = c * TL, (c + 1) * TL
        in_maps.append({
            "sta_loc": sta_loc[lo:hi],
            "pos_loc": pos_loc[lo:hi],
            "cnc_loc": cnc_loc[lo:hi],
            "eu_norm": eu_norm[lo:hi],
        })
    res = run_bass_kernel_spmd(nc, in_maps, core_ids=list(range(NCORES)))
    out = np.concatenate([r["ct"] for r in res.results], axis=0)
    return out


def make_timed_runner(nc, inputs):
    """Build a jitted runner (no donation, device-resident args) for timing.
    Returns fn() -> blocks until done."""
    import jax
    from jax.sharding import Mesh, PartitionSpec
    from jax.experimental.shard_map import shard_map
    from concourse.bass2jax import (_bass_exec_p, install_neuronx_cc_hook,
                                    partition_id_tensor)
    install_neuronx_cc_hook()

    partition_name = nc.partition_id_tensor.name if nc.partition_id_tensor else None
    in_names, out_names, out_avals, zero_outs = [], [], [], []
    for alloc in nc.m.functions[0].allocations:
        if not isinstance(alloc, mybir.MemoryLocationSet):
            continue
        name = alloc.memorylocations[0].name
        if alloc.kind == "ExternalInput":
            if name != partition_name:
                in_names.append(name)
        elif alloc.kind == "ExternalOutput":
            shape = tuple(alloc.tensor_shape)
            npdt = mybir.dt.np(alloc.dtype)
            out_names.append(name)
            out_avals.append(jax.core.ShapedArray(shape, npdt))
            zero_outs.append(np.zeros((NCORES * shape[0],) + shape[1:], npdt))
    n_params = len(in_names)
    all_names = in_names + out_names
    if partition_name is not None:
        all_names.append(partition_name)

    def _body(*args):
        operands = list(args)
        if partition_name is not None:
            operands.append(partition_id_tensor())
        outs = _bass_exec_p.bind(
            *operands, out_avals=tuple(out_avals), in_names=tuple(all_names),
            out_names=tuple(out_names), lowering_input_output_aliases=(),
            sim_require_finite=True, sim_require_nnan=True, nc=nc)
        return tuple(outs)

    devices = jax.devices()[:NCORES]
    mesh = Mesh(np.asarray(devices), ("core",))
    nin = n_params + len(out_names)
    fn = jax.jit(shard_map(_body, mesh=mesh,
                           in_specs=(PartitionSpec("core"),) * nin,
                           out_specs=(PartitionSpec("core"),) * len(out_names),
                           check_rep=False), keep_unused=True)
    concat_in = [np.concatenate([np.ascontiguousarray(
        inputs[name][c * TL:(c + 1) * TL]) for c in range(NCORES)], axis=0)
        for name in in_names]
    args = [jax.device_put(a) for a in concat_in + zero_outs]
    out = fn(*args)  # warm-up + compile
    jax.block_until_ready(out)

    def run():
        o = fn(*args)
        jax.block_until_ready(o)
        return o
    return run


def run_traced(inputs, trace=True):
    """For test.py: run with NTFF tracing, return (out, BassKernelResults)."""
    global _NC_CACHE
    if _NC_CACHE is None:
        _NC_CACHE = build_nc()
    nc = _NC_CACHE
    in_maps = []
    for c in range(NCORES):
        lo, hi = c * TL, (c + 1) * TL
        in_maps.append({k: np.ascontiguousarray(v[lo:hi]) for k, v in inputs.items()})
    res = run_bass_kernel_spmd(nc, in_maps, core_ids=list(range(NCORES)), trace=trace)
    out = np.concatenate([r["ct"] for r in res.results], axis=0)
    return out, res



# revision 11
# speedup vs baseline: 4.2844x; 4.2844x over previous
"""Trainium2 Bass kernel for nn_CritiGraph.

Math (vs the fp32 reference):
  dist(c1,c2,n) = sg * (1 - e/16) * n,  sg = sign(c1)*sign(c2),
  e = frexp_exp(|c1|^|c2| + 1) = bexp(float(|c1|^|c2|) + 1.5) - 126.
  ct[t,s,c,tp] = Q[t,s,tp] + M1[t,s,tp] * R[t,s,c,tp]
  where R = sgc * (e-16) (cnc sign applied via bf16 sign-bit xor),
        M1 = -sgp * norm / 128  (pos sign + /TP/16 folded),
        Q  = (sum_tp g - g)/8,  g = cos_sta_pos.

v2 layout: everything in natural (c,tp) order, S=128 on partitions.
Main loop per t (all full-width 2056-elem ops):
  p1  DVE TT : z16 = cmagR ^ pmag[s,tp] (bcast along c, stride-0 free dim)
  p2  ACT    : w = fp32(z16) + 1.5
  p3  DVE TS : sa = bf16((bits(w) >> 23) - 142)       [= e - 16, signed]
  p4  DVE TT : r16 = bits(sa) ^ sgn_mask (0x8000/0)
  p5  split by tp lane:
      lanes [0,NS):  ACT affine  out = M1*r + Q   (per-partition scale/bias)
      lanes [NS,8):  DVE TT r2 = r + Q2 (Q2 = Q/M1, bcast along c)
                     DVE TT out = r2 * M1 (bcast along c)
Sharding: T=128 rows split across 8 cores (16 rows each). Output gathered
on host (and upcast to fp32 if OUT_DT is bf16).
"""
import dataclasses
import numpy as np

import concourse.bass as bass
import concourse.mybir as mybir
from concourse import tile, bacc
from concourse.bass_utils import run_bass_kernel_spmd

dt = mybir.dt
Alu = mybir.AluOpType
Act = mybir.ActivationFunctionType

T, S, TP, C = 128, 128, 8, 257
NCORES = 8
TL = T // NCORES          # 16 t-rows per core
FD = C * TP               # 2056 free width, (c,tp) natural order

NS = 3                    # tp lanes [0,NS) handled by ACT affine slices
OUT_DT = "f32"            # 'f32' | 'bf16'


def _rep128(ap_row):
    """DRAM row AP -> same row broadcast to 128 partitions (stride-0)."""
    return dataclasses.replace(ap_row, ap=[[0, 128]] + list(ap_row.ap)[1:])


def _sub(ap_flat, off_elems, dims):
    """Offset a flat [P, N] AP by off_elems and install custom free dims."""
    s = ap_flat[:, off_elems:off_elems + 1]
    return dataclasses.replace(s, ap=[list(s.ap)[0]] + [list(d) for d in dims])


def build_nc(ns=None, out_dt=None):
    ns = NS if ns is None else ns
    out_dt = OUT_DT if out_dt is None else out_dt
    odt = dt.float32 if out_dt == "f32" else dt.bfloat16
    nc = bacc.Bacc("TRN2", target_bir_lowering=False, debug=False)

    sta_d = nc.dram_tensor("sta_loc", [TL, TP], dt.int32, kind="ExternalInput")
    pos_d = nc.dram_tensor("pos_loc", [TL, S, TP], dt.int32, kind="ExternalInput")
    cnc_d = nc.dram_tensor("cnc_loc", [TL, C, TP], dt.int32, kind="ExternalInput")
    norm_d = nc.dram_tensor("eu_norm", [TL, S], dt.float32, kind="ExternalInput")
    ct_d = nc.dram_tensor("ct", [TL, S, C, TP], odt, kind="ExternalOutput")

    with tile.TileContext(nc) as tc:
        with (
            tc.tile_pool(name="const", bufs=1) as cpool,
            tc.tile_pool(name="work", bufs=3) as wpool,
            tc.tile_pool(name="dram", bufs=1, space="DRAM") as dpool,
        ):
            # ---------------- preprocessing (small) ----------------
            # cnc -> (c,tp) magnitude / sign-mask planes, staged to DRAM
            cncraw = cpool.tile([TL, FD], dt.int32)
            nc.sync.dma_start(cncraw[:], cnc_d[:].rearrange("t c p -> t (c p)"))
            c_mag16 = cpool.tile([TL, FD], dt.uint16)
            c_sg0 = cpool.tile([TL, FD], dt.bfloat16)
            c_sgn = cpool.tile([TL, FD], dt.bfloat16)
            nc.scalar.activation(c_mag16[:], cncraw[:], Act.Abs)
            # sign as +-1 bf16: (cnc < 0) * -2 + 1
            nc.vector.tensor_scalar(c_sg0[:], cncraw[:], 0.0, -2.0,
                                    Alu.is_lt, Alu.mult)
            nc.vector.tensor_scalar(c_sgn[:], c_sg0[:], 1.0, None, Alu.add)
            d_cmag = dpool.tile([TL, FD], dt.uint16)
            d_csgn = dpool.tile([TL, FD], dt.bfloat16)
            nc.sync.dma_start(d_cmag[:], c_mag16[:])
            nc.sync.dma_start(d_csgn[:], c_sgn[:])

            # pos: [s, (t,tp)] in one strided DMA
            posraw = cpool.tile([S, TL * TP], dt.int32)
            pos_src = dataclasses.replace(
                pos_d[:].rearrange("t s p -> t (s p)").flatten(),
                ap=[[TP, S], [S * TP, TL], [1, TP]])
            nc.sync.dma_start(
                posraw[:].rearrange("s (t p) -> s t p", p=TP), pos_src)
            p_mag16 = cpool.tile([S, TL * TP], dt.uint16)
            nc.scalar.activation(p_mag16[:], posraw[:], Act.Abs)

            # norm[s, t] via strided DMA (4B gather, 8KB once)
            norm_sb = cpool.tile([S, TL], dt.float32)
            norm_src = dataclasses.replace(
                norm_d[:].flatten(), ap=[[1, S], [S, TL]])
            nc.sync.dma_start(norm_sb[:], norm_src)
            normB = dataclasses.replace(
                norm_sb[:], ap=[list(norm_sb[:].ap)[0], [1, TL], [0, TP]])
            # guarded norm (avoid 0/0 in Q2 = Q/M1)
            normg = cpool.tile([S, TL], dt.float32)
            nc.vector.tensor_scalar(normg[:], norm_sb[:], 1e-30, None, Alu.max)
            normgB = dataclasses.replace(
                normg[:], ap=[list(normg[:].ap)[0], [1, TL], [0, TP]])

            # M1[s,(t,tp)] = -sgp*norm/128 (exact: +-1/128 * norm)
            sgp2 = cpool.tile([S, TL * TP], dt.float32)
            nc.vector.tensor_scalar(sgp2[:], posraw[:], 0.0, 2.0, Alu.is_lt, Alu.mult)
            sgpm = cpool.tile([S, TL * TP], dt.float32)
            nc.vector.tensor_scalar(sgpm[:], sgp2[:], 1.0, 1.0 / 128, Alu.subtract, Alu.mult)
            M1 = cpool.tile([S, TL * TP], dt.float32)
            nc.vector.tensor_tensor(
                M1[:].rearrange("s (t p) -> s t p", p=TP),
                sgpm[:].rearrange("s (t p) -> s t p", p=TP),
                normgB, Alu.mult)

            # sta replicated to all partitions: [s, (t,tp)]
            starep = cpool.tile([S, TL * TP], dt.int32)
            sta_src = dataclasses.replace(
                sta_d[:].flatten(), ap=[[0, S], [1, TL * TP]])
            nc.sync.dma_start(starep[:], sta_src)

            # g path -> Q
            stamag = cpool.tile([S, TL * TP], dt.uint16)
            nc.scalar.activation(stamag[:], starep[:], Act.Abs)
            zg16 = cpool.tile([S, TL * TP], dt.uint16)
            nc.vector.tensor_tensor(zg16[:], stamag[:], p_mag16[:], Alu.bitwise_xor)
            wg = cpool.tile([S, TL * TP], dt.float32)
            nc.scalar.activation(wg[:], zg16[:], Act.Copy, bias=1.5, scale=1.0)
            eg32 = cpool.tile([S, TL * TP], dt.int32)
            nc.vector.tensor_scalar(eg32[:], wg[:].bitcast(dt.int32), 23, None,
                                    Alu.logical_shift_right)
            sag = cpool.tile([S, TL * TP], dt.bfloat16)
            nc.vector.tensor_scalar(sag[:], eg32[:], 142.0, None, Alu.subtract)
            sgxp = cpool.tile([S, TL * TP], dt.int32)
            nc.vector.tensor_tensor(sgxp[:], starep[:], posraw[:], Alu.bitwise_xor)
            sgx32 = cpool.tile([S, TL * TP], dt.int32)
            nc.vector.tensor_scalar(sgx32[:], sgxp[:], 16, 0x8000,
                                    Alu.logical_shift_right, Alu.bitwise_and)
            sgx16 = cpool.tile([S, TL * TP], dt.uint16)
            nc.vector.tensor_scalar(sgx16[:], sgx32[:], 1.0, None, Alu.mult)
            rg16 = cpool.tile([S, TL * TP], dt.uint16)
            nc.vector.tensor_tensor(rg16[:], sag[:].bitcast(dt.uint16), sgx16[:],
                                    Alu.bitwise_xor)
            t1 = cpool.tile([S, TL * TP], dt.float32)
            nc.vector.tensor_tensor(
                t1[:].rearrange("s (t p) -> s t p", p=TP),
                rg16[:].bitcast(dt.bfloat16).rearrange("s (t p) -> s t p", p=TP),
                normB, Alu.mult)
            t2 = cpool.tile([S, TL], dt.float32)
            nc.vector.tensor_reduce(t2[:].unsqueeze(2),
                                    t1[:].rearrange("s (t p) -> s t p", p=TP),
                                    axis=mybir.AxisListType.X, op=Alu.add)
            t2s = cpool.tile([S, TL], dt.float32)
            nc.vector.tensor_scalar(t2s[:], t2[:], 1.0 / 128, None, Alu.mult)
            Q = cpool.tile([S, TL * TP], dt.float32)
            nc.vector.scalar_tensor_tensor(
                Q[:].rearrange("s (t p) -> s t p", p=TP),
                t1[:].rearrange("s (t p) -> s t p", p=TP),
                1.0 / 128,
                t2s[:].unsqueeze(2).to_broadcast([S, TL, TP]),
                Alu.mult, Alu.subtract)

            # Q2 = Q/M1 (norm cancels algebraically; normg guards 0/0),
            # bf16 copies for the wide DVE p5 path
            recM1 = cpool.tile([S, TL * TP], dt.float32)
            nc.vector.reciprocal(recM1[:], M1[:])
            Q2 = cpool.tile([S, TL * TP], dt.float32)
            nc.vector.tensor_tensor(Q2[:], Q[:], recM1[:], Alu.mult)
            Q2b = cpool.tile([S, TL * TP], dt.bfloat16)
            nc.vector.tensor_scalar(Q2b[:], Q2[:], 0.0, None, Alu.add)
            M1b = cpool.tile([S, TL * TP], dt.bfloat16)
            nc.vector.tensor_scalar(M1b[:], M1[:], 0.0, None, Alu.add)

            # ---------------- main loop over t ----------------
            ND = TP - ns  # tp lanes on the DVE wide path
            for t in range(TL):
                cncR = wpool.tile([S, FD], dt.uint16, tag="cncR")
                scR = wpool.tile([S, FD], dt.bfloat16, tag="scR")
                nc.sync.dma_start(cncR[:], _rep128(d_cmag[t:t + 1, :]))
                nc.sync.dma_start(scR[:], _rep128(d_csgn[t:t + 1, :]))

                # p1: z = cmag ^ pmag (pmag bcast along c, tp innermost)
                z16 = wpool.tile([S, FD], dt.uint16, tag="z16")
                pm_b = _sub(p_mag16[:], t * TP, [[0, C], [1, TP]])
                nc.vector.tensor_tensor(
                    z16[:].rearrange("s (c p) -> s c p", p=TP),
                    cncR[:].rearrange("s (c p) -> s c p", p=TP),
                    pm_b, Alu.bitwise_xor)

                # p2: w = fp32(z) + 1.5 (exact frexp trick)
                w1 = wpool.tile([S, FD], dt.float32, tag="w1")
                nc.scalar.activation(w1[:], z16[:], Act.Copy, bias=1.5, scale=1.0)

                # p3: e32 = bits(w) >> 23 (biased exponent, in [127,143])
                e32 = wpool.tile([S, FD], dt.int32, tag="e32")
                nc.vector.tensor_scalar(e32[:], w1[:].bitcast(dt.int32), 23, None,
                                        Alu.logical_shift_right)

                # p4: r = (e32 - 142) * sgn  (= sgc * (e - 16), bf16)
                r16 = wpool.tile([S, FD], dt.bfloat16, tag="r16")
                nc.vector.scalar_tensor_tensor(r16[:], e32[:], 142.0, scR[:],
                                               Alu.subtract, Alu.mult)

                out_sb = wpool.tile([S, C, TP], odt, tag="out")
                out_flat = out_sb[:].rearrange("s c p -> s (c p)")

                # p5 ACT lanes [0, ns): out = M1*r + Q, exact fp32 affine
                for tp in range(ns):
                    rsl = _sub(r16[:], tp, [[TP, C]])
                    dsl = _sub(out_flat, tp, [[TP, C]])
                    nc.scalar.activation(dsl, rsl, Act.Identity,
                                         bias=Q[:, t * TP + tp: t * TP + tp + 1],
                                         scale=M1[:, t * TP + tp: t * TP + tp + 1])

                # p5 DVE lanes [ns, 8): r2 = r + Q2 ; out = r2 * M1
                if ND:
                    r2 = wpool.tile([S, C * ND], dt.bfloat16, tag="r2")
                    r16s = _sub(r16[:], ns, [[TP, C], [1, ND]])
                    q2_b = _sub(Q2b[:], t * TP + ns, [[0, C], [1, ND]])
                    nc.vector.tensor_tensor(
                        r2[:].rearrange("s (c p) -> s c p", p=ND), r16s, q2_b,
                        Alu.add)
                    m1_b = _sub(M1b[:], t * TP + ns, [[0, C], [1, ND]])
                    outs = _sub(out_flat, ns, [[TP, C], [1, ND]])
                    nc.vector.tensor_tensor(
                        outs, r2[:].rearrange("s (c p) -> s c p", p=ND), m1_b,
                        Alu.mult)

                nc.sync.dma_start(ct_d[t], out_sb[:])

    nc.compile()
    return nc


_NC_CACHE = None


def _get_nc():
    global _NC_CACHE
    if _NC_CACHE is None:
        _NC_CACHE = build_nc()
    return _NC_CACHE


def kernel(sta_loc, pos_loc, cnc_loc, eu_norm):
    nc = _get_nc()

    sta_loc = np.ascontiguousarray(np.asarray(sta_loc, dtype=np.int32))
    pos_loc = np.ascontiguousarray(np.asarray(pos_loc, dtype=np.int32))
    cnc_loc = np.ascontiguousarray(np.asarray(cnc_loc, dtype=np.int32))
    eu_norm = np.ascontiguousarray(np.asarray(eu_norm, dtype=np.float32))

    in_maps = []
    for c in range(NCORES):
        lo, hi = c * TL, (c + 1) * TL
        in_maps.append({
            "sta_loc": sta_loc[lo:hi],
            "pos_loc": pos_loc[lo:hi],
            "cnc_loc": cnc_loc[lo:hi],
            "eu_norm": eu_norm[lo:hi],
        })
    res = run_bass_kernel_spmd(nc, in_maps, core_ids=list(range(NCORES)))
    out = np.concatenate([r["ct"] for r in res.results], axis=0)
    return np.asarray(out, dtype=np.float32)


def run_traced(inputs, trace=True):
    """For test.py: run with NTFF tracing, return (out, BassKernelResults)."""
    nc = _get_nc()
    in_maps = []
    for c in range(NCORES):
        lo, hi = c * TL, (c + 1) * TL
        in_maps.append({k: np.ascontiguousarray(v[lo:hi]) for k, v in inputs.items()})
    res = run_bass_kernel_spmd(nc, in_maps, core_ids=list(range(NCORES)), trace=trace)
    out = np.concatenate([r["ct"] for r in res.results], axis=0)
    return np.asarray(out, dtype=np.float32), res


# revision 19
# speedup vs baseline: 4.7097x; 1.0993x over previous
"""Trainium2 Bass kernel for nn_CritiGraph.

Math (vs the fp32 reference):
  dist(c1,c2,n) = sg * (1 - e/16) * n,  sg = sign(c1)*sign(c2),
  e = frexp_exp(|c1|^|c2| + 1) = bexp(float(|c1|^|c2|) + 1.5) - 126.
  ct[t,s,c,tp] = Q[t,s,tp] + M1[t,s,tp] * R[t,s,c,tp]
  where R = sgc * (e-16) (cnc sign applied via bf16 sign-bit xor),
        M1 = -sgp * norm / 128  (pos sign + /TP/16 folded),
        Q  = (sum_tp g - g)/8,  g = cos_sta_pos.

v2 layout: everything in natural (c,tp) order, S=128 on partitions.
Main loop per t (all full-width 2056-elem ops):
  p1  DVE TT : z16 = cmagR ^ pmag[s,tp] (bcast along c, stride-0 free dim)
  p2  ACT    : w = fp32(z16) + 1.5
  p3  DVE TS : sa = bf16((bits(w) >> 23) - 142)       [= e - 16, signed]
  p4  DVE TT : r16 = bits(sa) ^ sgn_mask (0x8000/0)
  p5  split by tp lane:
      lanes [0,NS):  ACT affine  out = M1*r + Q   (per-partition scale/bias)
      lanes [NS,8):  DVE TT r2 = r + Q2 (Q2 = Q/M1, bcast along c)
                     DVE TT out = r2 * M1 (bcast along c)
Sharding: T=128 rows split across 8 cores (16 rows each). Output gathered
on host (and upcast to fp32 if OUT_DT is bf16).
"""
import dataclasses
import numpy as np

import concourse.bass as bass
import concourse.mybir as mybir
from concourse import tile, bacc
from concourse.bass_utils import run_bass_kernel_spmd

dt = mybir.dt
Alu = mybir.AluOpType
Act = mybir.ActivationFunctionType

T, S, TP, C = 128, 128, 8, 257
NCORES = 8
TL = T // NCORES          # 16 t-rows per core
FD = C * TP               # 2056 free width, (c,tp) natural order

NS = 1                    # tp lanes [0,NS) handled by ACT affine slices
OUT_DT = "bf16"           # 'f32' | 'bf16' (bf16 is upcast to f32 on host)


def _rep128(ap_row):
    """DRAM row AP -> same row broadcast to 128 partitions (stride-0)."""
    return dataclasses.replace(ap_row, ap=[[0, 128]] + list(ap_row.ap)[1:])


def _sub(ap_flat, off_elems, dims):
    """Offset a flat [P, N] AP by off_elems and install custom free dims."""
    s = ap_flat[:, off_elems:off_elems + 1]
    return dataclasses.replace(s, ap=[list(s.ap)[0]] + [list(d) for d in dims])


def build_nc(ns=None, out_dt=None):
    ns = NS if ns is None else ns
    out_dt = OUT_DT if out_dt is None else out_dt
    odt = dt.float32 if out_dt == "f32" else dt.bfloat16
    nc = bacc.Bacc("TRN2", target_bir_lowering=False, debug=False)

    sta_d = nc.dram_tensor("sta_loc", [TL, TP], dt.int32, kind="ExternalInput")
    pos_d = nc.dram_tensor("pos_loc", [TL, S, TP], dt.int32, kind="ExternalInput")
    cnc_d = nc.dram_tensor("cnc_loc", [TL, C, TP], dt.int32, kind="ExternalInput")
    norm_d = nc.dram_tensor("eu_norm", [TL, S], dt.float32, kind="ExternalInput")
    ct_d = nc.dram_tensor("ct", [TL, S, C, TP], odt, kind="ExternalOutput")

    with tile.TileContext(nc) as tc:
        with (
            tc.tile_pool(name="const", bufs=1) as cpool,
            tc.tile_pool(name="work", bufs=3) as wpool,
            tc.tile_pool(name="dram", bufs=1, space="DRAM") as dpool,
        ):
            # ---------------- preprocessing (small) ----------------
            # cnc -> (c,tp) magnitude / sign-mask planes, staged to DRAM
            cncraw = cpool.tile([TL, FD], dt.int32)
            nc.sync.dma_start(cncraw[:], cnc_d[:].rearrange("t c p -> t (c p)"))
            c_mag16 = cpool.tile([TL, FD], dt.uint16)
            c_sgn16 = cpool.tile([TL, FD], dt.uint16)
            nc.scalar.activation(c_mag16[:], cncraw[:], Act.Abs)
            # sign as bf16 sign-bit mask: 0x8000 if cnc < 0 else 0
            nc.vector.tensor_scalar(c_sgn16[:], cncraw[:], 0.0, 32768.0,
                                    Alu.is_lt, Alu.mult)
            d_cmag = dpool.tile([TL, FD], dt.uint16)
            d_csgn = dpool.tile([TL, FD], dt.uint16)
            nc.sync.dma_start(d_cmag[:], c_mag16[:])
            nc.sync.dma_start(d_csgn[:], c_sgn16[:])

            # pos: [s, (t,tp)] in one strided DMA
            posraw = cpool.tile([S, TL * TP], dt.int32)
            pos_src = dataclasses.replace(
                pos_d[:].rearrange("t s p -> t (s p)").flatten(),
                ap=[[TP, S], [S * TP, TL], [1, TP]])
            nc.sync.dma_start(
                posraw[:].rearrange("s (t p) -> s t p", p=TP), pos_src)
            p_mag16 = cpool.tile([S, TL * TP], dt.uint16)
            nc.scalar.activation(p_mag16[:], posraw[:], Act.Abs)

            # norm[s, t] via strided DMA (4B gather, 8KB once)
            norm_sb = cpool.tile([S, TL], dt.float32)
            norm_src = dataclasses.replace(
                norm_d[:].flatten(), ap=[[1, S], [S, TL]])
            nc.sync.dma_start(norm_sb[:], norm_src)
            normB = dataclasses.replace(
                norm_sb[:], ap=[list(norm_sb[:].ap)[0], [1, TL], [0, TP]])
            # guarded norm (avoid 0/0 in Q2 = Q/M1)
            normg = cpool.tile([S, TL], dt.float32)
            nc.vector.tensor_scalar(normg[:], norm_sb[:], 1e-30, None, Alu.max)
            normgB = dataclasses.replace(
                normg[:], ap=[list(normg[:].ap)[0], [1, TL], [0, TP]])

            # M1[s,(t,tp)] = -sgp*norm/128 (exact: +-1/128 * norm)
            sgp2 = cpool.tile([S, TL * TP], dt.float32)
            nc.vector.tensor_scalar(sgp2[:], posraw[:], 0.0, 2.0, Alu.is_lt, Alu.mult)
            sgpm = cpool.tile([S, TL * TP], dt.float32)
            nc.vector.tensor_scalar(sgpm[:], sgp2[:], 1.0, 1.0 / 128, Alu.subtract, Alu.mult)
            M1 = cpool.tile([S, TL * TP], dt.float32)
            nc.vector.tensor_tensor(
                M1[:].rearrange("s (t p) -> s t p", p=TP),
                sgpm[:].rearrange("s (t p) -> s t p", p=TP),
                normgB, Alu.mult)

            # sta replicated to all partitions: [s, (t,tp)]
            starep = cpool.tile([S, TL * TP], dt.int32)
            sta_src = dataclasses.replace(
                sta_d[:].flatten(), ap=[[0, S], [1, TL * TP]])
            nc.sync.dma_start(starep[:], sta_src)

            # g path -> Q
            stamag = cpool.tile([S, TL * TP], dt.uint16)
            nc.scalar.activation(stamag[:], starep[:], Act.Abs)
            zg16 = cpool.tile([S, TL * TP], dt.uint16)
            nc.vector.tensor_tensor(zg16[:], stamag[:], p_mag16[:], Alu.bitwise_xor)
            wg = cpool.tile([S, TL * TP], dt.float32)
            nc.scalar.activation(wg[:], zg16[:], Act.Copy, bias=1.5, scale=1.0)
            eg32 = cpool.tile([S, TL * TP], dt.int32)
            nc.vector.tensor_scalar(eg32[:], wg[:].bitcast(dt.int32), 23, None,
                                    Alu.logical_shift_right)
            sag = cpool.tile([S, TL * TP], dt.bfloat16)
            nc.vector.tensor_scalar(sag[:], eg32[:], 142.0, None, Alu.subtract)
            sgxp = cpool.tile([S, TL * TP], dt.int32)
            nc.vector.tensor_tensor(sgxp[:], starep[:], posraw[:], Alu.bitwise_xor)
            sgx32 = cpool.tile([S, TL * TP], dt.int32)
            nc.vector.tensor_scalar(sgx32[:], sgxp[:], 16, 0x8000,
                                    Alu.logical_shift_right, Alu.bitwise_and)
            sgx16 = cpool.tile([S, TL * TP], dt.uint16)
            nc.vector.tensor_scalar(sgx16[:], sgx32[:], 1.0, None, Alu.mult)
            rg16 = cpool.tile([S, TL * TP], dt.uint16)
            nc.vector.tensor_tensor(rg16[:], sag[:].bitcast(dt.uint16), sgx16[:],
                                    Alu.bitwise_xor)
            t1 = cpool.tile([S, TL * TP], dt.float32)
            nc.vector.tensor_tensor(
                t1[:].rearrange("s (t p) -> s t p", p=TP),
                rg16[:].bitcast(dt.bfloat16).rearrange("s (t p) -> s t p", p=TP),
                normB, Alu.mult)
            t2 = cpool.tile([S, TL], dt.float32)
            nc.vector.tensor_reduce(t2[:].unsqueeze(2),
                                    t1[:].rearrange("s (t p) -> s t p", p=TP),
                                    axis=mybir.AxisListType.X, op=Alu.add)
            t2s = cpool.tile([S, TL], dt.float32)
            nc.vector.tensor_scalar(t2s[:], t2[:], 1.0 / 128, None, Alu.mult)
            Q = cpool.tile([S, TL * TP], dt.float32)
            nc.vector.scalar_tensor_tensor(
                Q[:].rearrange("s (t p) -> s t p", p=TP),
                t1[:].rearrange("s (t p) -> s t p", p=TP),
                1.0 / 128,
                t2s[:].unsqueeze(2).to_broadcast([S, TL, TP]),
                Alu.mult, Alu.subtract)

            cm142 = cpool.tile([S, 1], dt.float32)
            nc.vector.memset(cm142[:], -142.0)

            # Q2 = Q/M1 (norm cancels algebraically; normg guards 0/0),
            # bf16 copies for the wide DVE p5 path
            recM1 = cpool.tile([S, TL * TP], dt.float32)
            nc.vector.reciprocal(recM1[:], M1[:])
            Q2 = cpool.tile([S, TL * TP], dt.float32)
            nc.vector.tensor_tensor(Q2[:], Q[:], recM1[:], Alu.mult)
            Q2b = cpool.tile([S, TL * TP], dt.bfloat16)
            nc.vector.tensor_scalar(Q2b[:], Q2[:], 0.0, None, Alu.add)
            M1b = cpool.tile([S, TL * TP], dt.bfloat16)
            nc.vector.tensor_scalar(M1b[:], M1[:], 0.0, None, Alu.add)

            # ---------------- main loop over t ----------------
            ND = TP - ns  # tp lanes on the DVE wide path
            for t in range(TL):
                cncR = wpool.tile([S, FD], dt.uint16, tag="cncR")
                scR = wpool.tile([S, FD], dt.uint16, tag="scR")
                nc.sync.dma_start(cncR[:], _rep128(d_cmag[t:t + 1, :]))
                nc.sync.dma_start(scR[:], _rep128(d_csgn[t:t + 1, :]))

                # p1: z = cmag ^ pmag, on packed u16 pairs (int32 xor)
                z16 = wpool.tile([S, FD], dt.uint16, tag="z16")
                pm_b = _sub(p_mag16[:].bitcast(dt.int32), t * TP // 2,
                            [[0, C], [1, TP // 2]])
                nc.vector.tensor_tensor(
                    z16[:].bitcast(dt.int32).rearrange("s (c p) -> s c p", p=TP // 2),
                    cncR[:].bitcast(dt.int32).rearrange("s (c p) -> s c p", p=TP // 2),
                    pm_b, Alu.bitwise_xor)

                # p2: w = fp32(z) + 1.5 (exact frexp trick)
                w1 = wpool.tile([S, FD], dt.float32, tag="w1")
                nc.scalar.activation(w1[:], z16[:], Act.Copy, bias=1.5, scale=1.0)

                # p3a: e32 = bits(w) >> 23 (biased exponent, in [127,143])
                e32 = wpool.tile([S, FD], dt.int32, tag="e32")
                nc.vector.tensor_scalar(e32[:], w1[:].bitcast(dt.int32), 23, None,
                                        Alu.logical_shift_right)

                # p3b (ACT): sa = bf16(e32 - 142) = e - 16, exact
                sa = wpool.tile([S, FD], dt.bfloat16, tag="sa")
                nc.scalar.activation(sa[:], e32[:], Act.Identity,
                                     bias=cm142[:], scale=1.0)

                # p4: r = sa ^ sign-mask, on packed u16 pairs (int32 xor)
                r16 = wpool.tile([S, FD], dt.bfloat16, tag="r16")
                nc.vector.tensor_tensor(r16[:].bitcast(dt.int32),
                                        sa[:].bitcast(dt.int32),
                                        scR[:].bitcast(dt.int32),
                                        Alu.bitwise_xor)

                out_sb = wpool.tile([S, C, TP], odt, tag="out")
                out_flat = out_sb[:].rearrange("s c p -> s (c p)")

                # p5 ACT lanes [0, ns): out = M1*r + Q, exact fp32 affine
                for tp in range(ns):
                    rsl = _sub(r16[:], tp, [[TP, C]])
                    dsl = _sub(out_flat, tp, [[TP, C]])
                    nc.scalar.activation(dsl, rsl, Act.Identity,
                                         bias=Q[:, t * TP + tp: t * TP + tp + 1],
                                         scale=M1[:, t * TP + tp: t * TP + tp + 1])

                # p5 DVE lanes [ns, 8): r2 = r + Q2 ; out = r2 * M1
                if ND:
                    r2 = wpool.tile([S, C * ND], dt.bfloat16, tag="r2")
                    r16s = _sub(r16[:], ns, [[TP, C], [1, ND]])
                    q2_b = _sub(Q2b[:], t * TP + ns, [[0, C], [1, ND]])
                    nc.vector.tensor_tensor(
                        r2[:].rearrange("s (c p) -> s c p", p=ND), r16s, q2_b,
                        Alu.add)
                    m1_b = _sub(M1b[:], t * TP + ns, [[0, C], [1, ND]])
                    outs = _sub(out_flat, ns, [[TP, C], [1, ND]])
                    nc.vector.tensor_tensor(
                        outs, r2[:].rearrange("s (c p) -> s c p", p=ND), m1_b,
                        Alu.mult)

                nc.sync.dma_start(ct_d[t], out_sb[:])

    nc.compile()
    return nc


_NC_CACHE = None


def _get_nc():
    global _NC_CACHE
    if _NC_CACHE is None:
        _NC_CACHE = build_nc()
    return _NC_CACHE


def kernel(sta_loc, pos_loc, cnc_loc, eu_norm):
    nc = _get_nc()

    sta_loc = np.ascontiguousarray(np.asarray(sta_loc, dtype=np.int32))
    pos_loc = np.ascontiguousarray(np.asarray(pos_loc, dtype=np.int32))
    cnc_loc = np.ascontiguousarray(np.asarray(cnc_loc, dtype=np.int32))
    eu_norm = np.ascontiguousarray(np.asarray(eu_norm, dtype=np.float32))

    in_maps = []
    for c in range(NCORES):
        lo, hi = c * TL, (c + 1) * TL
        in_maps.append({
            "sta_loc": sta_loc[lo:hi],
            "pos_loc": pos_loc[lo:hi],
            "cnc_loc": cnc_loc[lo:hi],
            "eu_norm": eu_norm[lo:hi],
        })
    res = run_bass_kernel_spmd(nc, in_maps, core_ids=list(range(NCORES)))
    out = np.concatenate([r["ct"] for r in res.results], axis=0)
    return np.asarray(out, dtype=np.float32)


def run_traced(inputs, trace=True):
    """For test.py: run with NTFF tracing, return (out, BassKernelResults)."""
    nc = _get_nc()
    in_maps = []
    for c in range(NCORES):
        lo, hi = c * TL, (c + 1) * TL
        in_maps.append({k: np.ascontiguousarray(v[lo:hi]) for k, v in inputs.items()})
    res = run_bass_kernel_spmd(nc, in_maps, core_ids=list(range(NCORES)), trace=trace)
    out = np.concatenate([r["ct"] for r in res.results], axis=0)
    return np.asarray(out, dtype=np.float32), res


# revision 22
# speedup vs baseline: 4.8282x; 1.0251x over previous
"""Trainium2 Bass kernel for nn_CritiGraph.

Math (vs the fp32 reference):
  dist(c1,c2,n) = sg * (1 - e/16) * n,  sg = sign(c1)*sign(c2),
  e = frexp_exp(|c1|^|c2| + 1) = bexp(float(|c1|^|c2|) + 1.5) - 126.
  ct[t,s,c,tp] = Q[t,s,tp] + M1[t,s,tp] * R[t,s,c,tp]
  where R = sgc * (e-16) (cnc sign applied via bf16 sign-bit xor),
        M1 = -sgp * norm / 128  (pos sign + /TP/16 folded),
        Q  = (sum_tp g - g)/8,  g = cos_sta_pos.

v2 layout: everything in natural (c,tp) order, S=128 on partitions.
Main loop per t (all full-width 2056-elem ops):
  p1  DVE TT : z16 = cmagR ^ pmag[s,tp] (bcast along c, stride-0 free dim)
  p2  ACT    : w = fp32(z16) + 1.5
  p3  DVE TS : sa = bf16((bits(w) >> 23) - 142)       [= e - 16, signed]
  p4  DVE TT : r16 = bits(sa) ^ sgn_mask (0x8000/0)
  p5  split by tp lane:
      lanes [0,NS):  ACT affine  out = M1*r + Q   (per-partition scale/bias)
      lanes [NS,8):  DVE TT r2 = r + Q2 (Q2 = Q/M1, bcast along c)
                     DVE TT out = r2 * M1 (bcast along c)
Sharding: T=128 rows split across 8 cores (16 rows each). Output gathered
on host (and upcast to fp32 if OUT_DT is bf16).
"""
import dataclasses
import numpy as np

import concourse.bass as bass
import concourse.mybir as mybir
from concourse import tile, bacc
from concourse.bass_utils import run_bass_kernel_spmd

dt = mybir.dt
Alu = mybir.AluOpType
Act = mybir.ActivationFunctionType

T, S, TP, C = 128, 128, 8, 257
NCORES = 8
TL = T // NCORES          # 16 t-rows per core
FD = C * TP               # 2056 free width, (c,tp) natural order

NS = 2                    # tp lanes [0,NS) handled by ACT affine slices
OUT_DT = "bf16"           # 'f32' | 'bf16' (bf16 is upcast to f32 on host)


def _rep128(ap_row):
    """DRAM row AP -> same row broadcast to 128 partitions (stride-0)."""
    return dataclasses.replace(ap_row, ap=[[0, 128]] + list(ap_row.ap)[1:])


def _sub(ap_flat, off_elems, dims):
    """Offset a flat [P, N] AP by off_elems and install custom free dims."""
    s = ap_flat[:, off_elems:off_elems + 1]
    return dataclasses.replace(s, ap=[list(s.ap)[0]] + [list(d) for d in dims])


def build_nc(ns=None, out_dt=None):
    ns = NS if ns is None else ns
    out_dt = OUT_DT if out_dt is None else out_dt
    odt = dt.float32 if out_dt == "f32" else dt.bfloat16
    nc = bacc.Bacc("TRN2", target_bir_lowering=False, debug=False)

    sta_d = nc.dram_tensor("sta_loc", [TL, TP], dt.int32, kind="ExternalInput")
    pos_d = nc.dram_tensor("pos_loc", [TL, S, TP], dt.int32, kind="ExternalInput")
    cnc_d = nc.dram_tensor("cnc_loc", [TL, C, TP], dt.int32, kind="ExternalInput")
    norm_d = nc.dram_tensor("eu_norm", [TL, S], dt.float32, kind="ExternalInput")
    ct_d = nc.dram_tensor("ct", [TL, S, C, TP], odt, kind="ExternalOutput")

    with tile.TileContext(nc) as tc:
        with (
            tc.tile_pool(name="const", bufs=1) as cpool,
            tc.tile_pool(name="work", bufs=3) as wpool,
            tc.tile_pool(name="dram", bufs=1, space="DRAM") as dpool,
        ):
            # ---------------- preprocessing (small) ----------------
            # cnc -> (c,tp) magnitude / sign-mask planes, staged to DRAM
            cncraw = cpool.tile([TL, FD], dt.int32)
            nc.sync.dma_start(cncraw[:], cnc_d[:].rearrange("t c p -> t (c p)"))
            c_mag16 = cpool.tile([TL, FD], dt.uint16)
            c_sgn16 = cpool.tile([TL, FD], dt.uint16)
            nc.scalar.activation(c_mag16[:], cncraw[:], Act.Abs)
            # sign as bf16 sign-bit mask: 0x8000 if cnc < 0 else 0
            nc.vector.tensor_scalar(c_sgn16[:], cncraw[:], 0.0, 32768.0,
                                    Alu.is_lt, Alu.mult)
            d_cmag = dpool.tile([TL, FD], dt.uint16)
            d_csgn = dpool.tile([TL, FD], dt.uint16)
            nc.sync.dma_start(d_cmag[:], c_mag16[:])
            nc.sync.dma_start(d_csgn[:], c_sgn16[:])

            # pos: [s, (t,tp)] in one strided DMA
            posraw = cpool.tile([S, TL * TP], dt.int32)
            pos_src = dataclasses.replace(
                pos_d[:].rearrange("t s p -> t (s p)").flatten(),
                ap=[[TP, S], [S * TP, TL], [1, TP]])
            nc.sync.dma_start(
                posraw[:].rearrange("s (t p) -> s t p", p=TP), pos_src)
            p_mag16 = cpool.tile([S, TL * TP], dt.uint16)
            nc.scalar.activation(p_mag16[:], posraw[:], Act.Abs)

            # norm[s, t] via strided DMA (4B gather, 8KB once)
            norm_sb = cpool.tile([S, TL], dt.float32)
            norm_src = dataclasses.replace(
                norm_d[:].flatten(), ap=[[1, S], [S, TL]])
            nc.sync.dma_start(norm_sb[:], norm_src)
            normB = dataclasses.replace(
                norm_sb[:], ap=[list(norm_sb[:].ap)[0], [1, TL], [0, TP]])
            # guarded norm (avoid 0/0 in Q2 = Q/M1)
            normg = cpool.tile([S, TL], dt.float32)
            nc.vector.tensor_scalar(normg[:], norm_sb[:], 1e-30, None, Alu.max)
            normgB = dataclasses.replace(
                normg[:], ap=[list(normg[:].ap)[0], [1, TL], [0, TP]])

            # M1[s,(t,tp)] = -sgp*norm/128 (exact: +-1/128 * norm)
            sgp2 = cpool.tile([S, TL * TP], dt.float32)
            nc.vector.tensor_scalar(sgp2[:], posraw[:], 0.0, 2.0, Alu.is_lt, Alu.mult)
            sgpm = cpool.tile([S, TL * TP], dt.float32)
            nc.vector.tensor_scalar(sgpm[:], sgp2[:], 1.0, 1.0 / 128, Alu.subtract, Alu.mult)
            M1 = cpool.tile([S, TL * TP], dt.float32)
            nc.vector.tensor_tensor(
                M1[:].rearrange("s (t p) -> s t p", p=TP),
                sgpm[:].rearrange("s (t p) -> s t p", p=TP),
                normgB, Alu.mult)

            # sta replicated to all partitions: [s, (t,tp)]
            starep = cpool.tile([S, TL * TP], dt.int32)
            sta_src = dataclasses.replace(
                sta_d[:].flatten(), ap=[[0, S], [1, TL * TP]])
            nc.sync.dma_start(starep[:], sta_src)

            # g path -> Q
            stamag = cpool.tile([S, TL * TP], dt.uint16)
            nc.scalar.activation(stamag[:], starep[:], Act.Abs)
            zg16 = cpool.tile([S, TL * TP], dt.uint16)
            nc.vector.tensor_tensor(zg16[:], stamag[:], p_mag16[:], Alu.bitwise_xor)
            wg = cpool.tile([S, TL * TP], dt.float32)
            nc.scalar.activation(wg[:], zg16[:], Act.Copy, bias=1.5, scale=1.0)
            eg32 = cpool.tile([S, TL * TP], dt.int32)
            nc.vector.tensor_scalar(eg32[:], wg[:].bitcast(dt.int32), 23, None,
                                    Alu.logical_shift_right)
            sag = cpool.tile([S, TL * TP], dt.bfloat16)
            nc.vector.tensor_scalar(sag[:], eg32[:], 142.0, None, Alu.subtract)
            sgxp = cpool.tile([S, TL * TP], dt.int32)
            nc.vector.tensor_tensor(sgxp[:], starep[:], posraw[:], Alu.bitwise_xor)
            sgx32 = cpool.tile([S, TL * TP], dt.int32)
            nc.vector.tensor_scalar(sgx32[:], sgxp[:], 16, 0x8000,
                                    Alu.logical_shift_right, Alu.bitwise_and)
            sgx16 = cpool.tile([S, TL * TP], dt.uint16)
            nc.vector.tensor_scalar(sgx16[:], sgx32[:], 1.0, None, Alu.mult)
            rg16 = cpool.tile([S, TL * TP], dt.uint16)
            nc.vector.tensor_tensor(rg16[:], sag[:].bitcast(dt.uint16), sgx16[:],
                                    Alu.bitwise_xor)
            t1 = cpool.tile([S, TL * TP], dt.float32)
            nc.vector.tensor_tensor(
                t1[:].rearrange("s (t p) -> s t p", p=TP),
                rg16[:].bitcast(dt.bfloat16).rearrange("s (t p) -> s t p", p=TP),
                normB, Alu.mult)
            t2 = cpool.tile([S, TL], dt.float32)
            nc.vector.tensor_reduce(t2[:].unsqueeze(2),
                                    t1[:].rearrange("s (t p) -> s t p", p=TP),
                                    axis=mybir.AxisListType.X, op=Alu.add)
            t2s = cpool.tile([S, TL], dt.float32)
            nc.vector.tensor_scalar(t2s[:], t2[:], 1.0 / 128, None, Alu.mult)
            Q = cpool.tile([S, TL * TP], dt.float32)
            nc.vector.scalar_tensor_tensor(
                Q[:].rearrange("s (t p) -> s t p", p=TP),
                t1[:].rearrange("s (t p) -> s t p", p=TP),
                1.0 / 128,
                t2s[:].unsqueeze(2).to_broadcast([S, TL, TP]),
                Alu.mult, Alu.subtract)

            # Q2 = Q/M1 (norm cancels algebraically; normg guards 0/0),
            # bf16 copies for the wide DVE p5 path
            recM1 = cpool.tile([S, TL * TP], dt.float32)
            nc.vector.reciprocal(recM1[:], M1[:])
            Q2 = cpool.tile([S, TL * TP], dt.float32)
            nc.vector.tensor_tensor(Q2[:], Q[:], recM1[:], Alu.mult)
            Q2b = cpool.tile([S, TL * TP], dt.bfloat16)
            nc.vector.tensor_scalar(Q2b[:], Q2[:], 0.0, None, Alu.add)
            M1b = cpool.tile([S, TL * TP], dt.bfloat16)
            nc.vector.tensor_scalar(M1b[:], M1[:], 0.0, None, Alu.add)

            # ---------------- main loop over t ----------------
            ND = TP - ns  # tp lanes on the DVE wide path
            for t in range(TL):
                cncR = wpool.tile([S, FD], dt.uint16, tag="cncR")
                scR = wpool.tile([S, FD], dt.uint16, tag="scR")
                nc.sync.dma_start(cncR[:], _rep128(d_cmag[t:t + 1, :]))
                nc.sync.dma_start(scR[:], _rep128(d_csgn[t:t + 1, :]))

                # p1: z = cmag ^ pmag, on packed u16 pairs (int32 xor)
                z16 = wpool.tile([S, FD], dt.uint16, tag="z16")
                pm_b = _sub(p_mag16[:].bitcast(dt.int32), t * TP // 2,
                            [[0, C], [1, TP // 2]])
                nc.vector.tensor_tensor(
                    z16[:].bitcast(dt.int32).rearrange("s (c p) -> s c p", p=TP // 2),
                    cncR[:].bitcast(dt.int32).rearrange("s (c p) -> s c p", p=TP // 2),
                    pm_b, Alu.bitwise_xor)

                # p2: w = fp32(z) + 1.5 (exact frexp trick)
                w1 = wpool.tile([S, FD], dt.float32, tag="w1")
                nc.scalar.activation(w1[:], z16[:], Act.Copy, bias=1.5, scale=1.0)

                # p3a: e16 = hi16(bits(w)) >> 7 (biased exponent, u16 contiguous)
                e16 = wpool.tile([S, FD], dt.uint16, tag="e16")
                w_hi = _sub(w1[:].bitcast(dt.uint16), 1, [[2, FD]])
                nc.vector.tensor_scalar(e16[:], w_hi, 7, None,
                                        Alu.logical_shift_right)

                # p3b: sa = bf16(e16 - 142) = e - 16, exact (2-byte fast path)
                sa = wpool.tile([S, FD], dt.bfloat16, tag="sa")
                nc.vector.tensor_scalar(sa[:], e16[:], 142.0, None, Alu.subtract)

                # p4: r = sa ^ sign-mask, on packed u16 pairs (int32 xor)
                r16 = wpool.tile([S, FD], dt.bfloat16, tag="r16")
                nc.vector.tensor_tensor(r16[:].bitcast(dt.int32),
                                        sa[:].bitcast(dt.int32),
                                        scR[:].bitcast(dt.int32),
                                        Alu.bitwise_xor)

                out_sb = wpool.tile([S, C, TP], odt, tag="out")
                out_flat = out_sb[:].rearrange("s c p -> s (c p)")

                # p5 ACT lanes [0, ns): out = M1*r + Q, exact fp32 affine
                for tp in range(ns):
                    rsl = _sub(r16[:], tp, [[TP, C]])
                    dsl = _sub(out_flat, tp, [[TP, C]])
                    nc.scalar.activation(dsl, rsl, Act.Identity,
                                         bias=Q[:, t * TP + tp: t * TP + tp + 1],
                                         scale=M1[:, t * TP + tp: t * TP + tp + 1])

                # p5 DVE lanes [ns, 8): r2 = r + Q2 ; out = r2 * M1
                if ND:
                    r2 = wpool.tile([S, C * ND], dt.bfloat16, tag="r2")
                    r16s = _sub(r16[:], ns, [[TP, C], [1, ND]])
                    q2_b = _sub(Q2b[:], t * TP + ns, [[0, C], [1, ND]])
                    nc.vector.tensor_tensor(
                        r2[:].rearrange("s (c p) -> s c p", p=ND), r16s, q2_b,
                        Alu.add)
                    m1_b = _sub(M1b[:], t * TP + ns, [[0, C], [1, ND]])
                    outs = _sub(out_flat, ns, [[TP, C], [1, ND]])
                    nc.vector.tensor_tensor(
                        outs, r2[:].rearrange("s (c p) -> s c p", p=ND), m1_b,
                        Alu.mult)

                nc.sync.dma_start(ct_d[t], out_sb[:])

    nc.compile()
    return nc


_NC_CACHE = None


def _get_nc():
    global _NC_CACHE
    if _NC_CACHE is None:
        _NC_CACHE = build_nc()
    return _NC_CACHE


def kernel(sta_loc, pos_loc, cnc_loc, eu_norm):
    nc = _get_nc()

    sta_loc = np.ascontiguousarray(np.asarray(sta_loc, dtype=np.int32))
    pos_loc = np.ascontiguousarray(np.asarray(pos_loc, dtype=np.int32))
    cnc_loc = np.ascontiguousarray(np.asarray(cnc_loc, dtype=np.int32))
    eu_norm = np.ascontiguousarray(np.asarray(eu_norm, dtype=np.float32))

    in_maps = []
    for c in range(NCORES):
        lo, hi = c * TL, (c + 1) * TL
        in_maps.append({
            "sta_loc": sta_loc[lo:hi],
            "pos_loc": pos_loc[lo:hi],
            "cnc_loc": cnc_loc[lo:hi],
            "eu_norm": eu_norm[lo:hi],
        })
    res = run_bass_kernel_spmd(nc, in_maps, core_ids=list(range(NCORES)))
    out = np.concatenate([r["ct"] for r in res.results], axis=0)
    return np.asarray(out, dtype=np.float32)


def run_traced(inputs, trace=True):
    """For test.py: run with NTFF tracing, return (out, BassKernelResults)."""
    nc = _get_nc()
    in_maps = []
    for c in range(NCORES):
        lo, hi = c * TL, (c + 1) * TL
        in_maps.append({k: np.ascontiguousarray(v[lo:hi]) for k, v in inputs.items()})
    res = run_bass_kernel_spmd(nc, in_maps, core_ids=list(range(NCORES)), trace=trace)
    out = np.concatenate([r["ct"] for r in res.results], axis=0)
    return np.asarray(out, dtype=np.float32), res
